# revision 24
# baseline (speedup 1.0000x reference)
"""MultiHeadProductKeyRouter Trainium2 Bass kernel.

Math: reference computes
    s1 = x @ W1.T  -> (T, H*SK) -> (T, H, SK)
    s2 = x @ W2.T
    head_scores[t,h,i*SK+j] = s1[t,h,i] + s2[t,h,j]
    scores = head_scores.mean(axis=-2)          # (T, SK*SK)
    topk_vals, topk_idx = top_k(scores, 8); gates = softmax(topk_vals)

Since the mean over heads commutes with the outer-sum:
    scores[t, i*SK+j] = a1[t,i] + a2[t,j]
with a1 = mean_h s1, a2 = mean_h s2, i.e. a = x @ Vall.T where
Vall = [mean_h W1_heads ; mean_h W2_heads]  (128 x 2048).

scores is produced on the TensorEngine as A @ M where A = [a1|a2] (128
features per token) and M is a constant 128x4096 0/1 matrix with
M[i, i*64+j] = 1 and M[64+j, i*64+j] = 1.  Each PSUM element is then a
single fp32 add a1[i]+a2[j] (zeros accumulate exactly), bit-identical to
the candidate sums used for top-k.

Top-8 of the outer sum: any top-8 element must have i in top8(a1) and
j in top8(a2), so top-8 over the 64 candidate sums of top8(a1) x top8(a2)
equals top-8 of the full 4096 row.  DVE max/max_index provide top-8
values+indices directly; candidate position is decoded as (p,q) = (c//8,
c%8) and mapped through the top-8 index lists.

Sharding: data parallel over tokens, 16384 tokens -> 8 cores x 2048.
"""

import os
import numpy as np

import concourse.bass as bass
import concourse.mybir as mybir
import concourse.tile as tile
from concourse import bass_utils

F32 = mybir.dt.float32
U32 = mybir.dt.uint32
I32 = mybir.dt.int32
Alu = mybir.AluOpType

D = 2048          # model dim
SK = 64           # sqrt(num experts)
KOUT = SK * SK    # 4096
NF = 128          # features after head-mean: 64 (a1) + 64 (a2)
TOPK = 8
DC = D // 128     # 16 contraction chunks
N_CORES = 8
TOK_TOTAL = 4 * 4096
TOK_PER_CORE = TOK_TOTAL // N_CORES   # 2048
GROUP = 512       # tokens per matmul1 group
SUB = GROUP // 128


# ---------------------------------------------------------------- host consts
def make_consts():
    # Head-sum selectors: W tiles are [128 rows = 2 heads x 64] ; out column
    # (i or 64+j) accumulates 0.25 * row (p % 64).
    hsel1 = np.zeros((128, 128), np.float32)
    hsel2 = np.zeros((128, 128), np.float32)
    for p in range(128):
        hsel1[p, p % SK] = 0.25
        hsel2[p, SK + (p % SK)] = 0.25
    ident = np.eye(128, dtype=np.float32)
    return {"hsel1": hsel1, "hsel2": hsel2, "ident": ident}


# Number of trailing i-blocks (of 8 rows of 64 scores each) of the outer-sum
# handed to GpSimd per subtile; rest goes to DVE.
GPSIMD_IBLOCKS = 2


def _bc_outer(a, m):
    """[128, n] -> [128, n, m] view repeating each element (stride-0 inner)."""
    return bass.AP(tensor=a.tensor, offset=a.offset,
                   ap=[a.ap[0], a.ap[1], [0, m]])


def _bc_inner(a, n):
    """[128, m] -> [128, n, m] view repeating the row n times (stride-0 outer)."""
    return bass.AP(tensor=a.tensor, offset=a.offset,
                   ap=[a.ap[0], [0, n], a.ap[1]])


def _r3(sc, m):
    return sc.rearrange("p (i j) -> p i j", j=m)


# ---------------------------------------------------------------- tile kernel
def router_tile_kernel(tc, outs, ins, n_tokens, ctx):
    nc = tc.nc
    n_groups = n_tokens // GROUP

    x = ins["x"]            # [n_tokens, D]
    w1 = ins["w1"]          # [256, D]
    w2 = ins["w2"]          # [256, D]
    hsel1 = ins["hsel1"]    # [128, 128]
    hsel2 = ins["hsel2"]    # [128, 128]
    ident = ins["ident"]    # [128, 128]
    scores_d = outs["scores"]   # [n_tokens, KOUT] f32
    idx_d = outs["idx"]         # [n_tokens, TOPK] i32
    gates_d = outs["gates"]     # [n_tokens, TOPK] f32

    consts = ctx.enter_context(tc.tile_pool(name="consts", bufs=1))
    wpool = ctx.enter_context(tc.tile_pool(name="wpool", bufs=2))
    xpool = ctx.enter_context(tc.tile_pool(name="xpool", bufs=2))
    xtpool = ctx.enter_context(tc.tile_pool(name="xtpool", bufs=2))
    apool = ctx.enter_context(tc.tile_pool(name="apool", bufs=2))
    atpool = ctx.enter_context(tc.tile_pool(name="atpool", bufs=2))
    spool = ctx.enter_context(tc.tile_pool(name="spool", bufs=3))
    small = ctx.enter_context(tc.tile_pool(name="small", bufs=3))
    ps_tr = ctx.enter_context(tc.tile_pool(name="ps_tr", bufs=4, space="PSUM"))
    ps_mm1 = ctx.enter_context(tc.tile_pool(name="ps_mm1", bufs=2, space="PSUM"))
    ps_at = ctx.enter_context(tc.tile_pool(name="ps_at", bufs=2, space="PSUM"))

    # ---- constants into SBUF
    hs1_sb = consts.tile([128, 128], F32)
    nc.sync.dma_start(hs1_sb, hsel1)
    hs2_sb = consts.tile([128, 128], F32)
    nc.sync.dma_start(hs2_sb, hsel2)
    id_sb = consts.tile([128, 128], F32)
    nc.sync.dma_start(id_sb, ident)

    # ---- Vall = 0.25 * head-sum of [W1;W2]  -> [128 f, D]
    vall_sb = consts.tile([128, D], F32)
    pv = [ps_tr.tile([128, 512], F32, tag="tr", name=f"pv{c4}")
          for c4 in range(D // 512)]
    for i, (w, hs) in enumerate(
            ((w1, hs1_sb), (w1, hs1_sb), (w2, hs2_sb), (w2, hs2_sb))):
        r = i % 2
        wt = wpool.tile([128, D], F32, tag="w")
        nc.sync.dma_start(wt, w[r * 128:(r + 1) * 128, :])
        for c4 in range(D // 512):
            nc.tensor.matmul(pv[c4], hs, wt[:, c4 * 512:(c4 + 1) * 512],
                             start=(i == 0), stop=(i == 3))
    for c4 in range(D // 512):
        nc.vector.tensor_copy(vall_sb[:, c4 * 512:(c4 + 1) * 512], pv[c4])

    # ---- VallT chunks [128 d, 128 f] for matmul1 stationary
    vallT_sb = consts.tile([128, DC, 128], F32)
    for c4 in range(DC // 4):
        pt = ps_tr.tile([128, 512], F32, tag="tr")
        for cc in range(4):
            c = c4 * 4 + cc
            nc.tensor.transpose(pt[:, cc * 128:(cc + 1) * 128],
                                vall_sb[:, c * 128:(c + 1) * 128], id_sb)
        nc.vector.tensor_copy(vallT_sb[:, c4 * 4:(c4 + 1) * 4, :], pt)

    # ---- main loop over token groups
    for g in range(n_groups):
        xt = xtpool.tile([128, DC, GROUP], F32, tag="xt")  # [d-chunk part, c, t]
        for s in range(SUB):
            xs = xpool.tile([128, D], F32, tag="x")
            nc.sync.dma_start(xs, x[(g * SUB + s) * 128:(g * SUB + s + 1) * 128, :])
            for c4 in range(DC // 4):
                pt = ps_tr.tile([128, 512], F32, tag="tr")
                for cc in range(4):
                    c = c4 * 4 + cc
                    nc.tensor.transpose(pt[:, cc * 128:(cc + 1) * 128],
                                        xs[:, c * 128:(c + 1) * 128], id_sb)
                # one strided copy: psum [128,4,128] -> xt[:, 4c4:4c4+4, s*128:+128]
                dst = xt[:, c4 * 4:(c4 + 1) * 4, s * 128:(s + 1) * 128]
                nc.scalar.copy(dst, pt.rearrange("p (c t) -> p c t", c=4))

        # matmul1: A[f, t] = sum_c VallT[c].T @ xt[c]
        a_ps = ps_mm1.tile([128, GROUP], F32, tag="mm1")
        for c in range(DC):
            nc.tensor.matmul(a_ps, vallT_sb[:, c, :], xt[:, c, :],
                             start=(c == 0), stop=(c == DC - 1))
        a_sb = apool.tile([128, GROUP], F32, tag="a")
        nc.scalar.copy(a_sb, a_ps)

        # At[t, f] per subtile (for top-k)
        at_ps = ps_at.tile([128, SUB, 128], F32, tag="at")
        for s in range(SUB):
            nc.tensor.transpose(at_ps[:, s, :], a_sb[:, s * 128:(s + 1) * 128], id_sb)
        at_sb = atpool.tile([128, SUB, 128], F32, tag="at")
        nc.scalar.copy(at_sb, at_ps)

        # scores outer-sum + topk per subtile
        for s in range(SUB):
            tok0 = (g * SUB + s) * 128
            at = at_sb[:, s, :]
            a1 = at[:, 0:SK]
            a2 = at[:, SK:2 * SK]
            sc_sb = spool.tile([128, KOUT], F32, tag="sc")
            # scores[t, i*64+j] = a1[t,i] + a2[t,j] via stride-0 broadcast APs
            if GPSIMD_IBLOCKS:
                split = SK - GPSIMD_IBLOCKS * 8
                nc.vector.tensor_add(
                    _r3(sc_sb[:, :split * SK], SK),
                    _bc_outer(a1[:, :split], SK), _bc_inner(a2, split))
                nc.gpsimd.tensor_add(
                    _r3(sc_sb[:, split * SK:], SK),
                    _bc_outer(a1[:, split:], SK), _bc_inner(a2, SK - split))
            else:
                nc.vector.tensor_add(
                    _r3(sc_sb, SK), _bc_outer(a1, SK), _bc_inner(a2, SK))
            nc.sync.dma_start(scores_d[tok0:tok0 + 128, :], sc_sb)

            _topk_block(nc, small, at, idx_d, gates_d, tok0)


def _topk_block(nc, small, at, idx_d, gates_d, tok0):
    """Top-8 + gates for one 128-token subtile. at: [128 t, 128 f] SBUF."""
    a1 = at[:, 0:SK]
    a2 = at[:, SK:2 * SK]
    t8 = [128, TOPK]

    v1 = small.tile(t8, F32, tag="v1")
    v2 = small.tile(t8, F32, tag="v2")
    i1u = small.tile(t8, U32, tag="i1u")
    i2u = small.tile(t8, U32, tag="i2u")
    i1f = small.tile(t8, F32, tag="i1f")
    i2f = small.tile(t8, F32, tag="i2f")
    i1n = small.tile(t8, F32, tag="i1n")
    cv = small.tile([128, 64], F32, tag="cv")
    cin = small.tile([128, 64], F32, tag="cin")
    tv = small.tile(t8, F32, tag="tv")
    eqm = small.tile([128, TOPK, 64], F32, tag="eqm")
    red = small.tile(t8, F32, tag="red")
    flatf = small.tile(t8, F32, tag="flatf")
    idx_sb = small.tile(t8, I32, tag="idx_sb")
    negm = small.tile([128, 1], F32, tag="negm")
    e8 = small.tile(t8, F32, tag="e8")
    ssum = small.tile([128, 1], F32, tag="ssum")
    rinv = small.tile([128, 1], F32, tag="rinv")
    gates_sb = small.tile(t8, F32, tag="gates_sb")

    # top-8 of each half (values sorted desc; indices = first occurrence)
    nc.vector.max(v1, a1)
    nc.vector.max_index(i1u, v1, a1)
    nc.vector.max(v2, a2)
    nc.vector.max_index(i2u, v2, a2)
    nc.vector.tensor_copy(i1f, i1u)
    nc.vector.tensor_copy(i2f, i2u)

    # 64 candidates cv[p*8+q] = v1[p] + v2[q]; the exact same fp32 adds as
    # the scores outer-sum, so values are bit-identical to scores entries.
    nc.vector.tensor_add(_r3(cv, 8), _bc_outer(v1, 8), _bc_inner(v2, 8))
    # negated flat index per candidate: cin[p*8+q] = 4096 - (64*i1[p] + i2[q])
    nc.vector.tensor_scalar(i1n, i1f, -64.0, 4096.0, op0=Alu.mult, op1=Alu.add)
    nc.vector.tensor_tensor(_r3(cin, 8), _bc_outer(i1n, 8), _bc_inner(i2f, 8),
                            op=Alu.subtract)

    nc.vector.max(tv, cv)

    # flat idx of the k-th winner: match tv[k] against cv, pick the matching
    # candidate's min flat idx (max of cin), all in one masked reduce.
    nc.vector.tensor_tensor(eqm, _bc_inner(cv, 8),
                            _bc_outer(tv, 64), op=Alu.is_equal)
    nc.vector.tensor_mul(eqm, eqm, _bc_inner(cin, 8))
    nc.vector.reduce_max(red, eqm, axis=mybir.AxisListType.X)
    nc.vector.tensor_scalar(flatf, red, -1.0, 4096.0, op0=Alu.mult, op1=Alu.add)
    nc.vector.tensor_copy(idx_sb, flatf)

    # gates = softmax(tv) (tv[:,0] is the max)
    nc.vector.tensor_scalar_mul(negm, tv[:, 0:1], -1.0)
    nc.scalar.activation(e8, tv, mybir.ActivationFunctionType.Exp,
                         bias=negm, scale=1.0, accum_out=ssum)
    nc.vector.reciprocal(rinv, ssum)
    nc.vector.tensor_scalar_mul(gates_sb, e8, rinv)

    nc.sync.dma_start(idx_d[tok0:tok0 + 128, :], idx_sb)
    nc.sync.dma_start(gates_d[tok0:tok0 + 128, :], gates_sb)


def _split_multi_waits(nc, limit=1):
    """Walrus in this toolchain rejects instructions carrying more than one
    semaphore wait (fp32 Matmult LDW path asserts at even 2, end-of-kernel
    drains at 5).  Post-process the scheduled BIR: move all but `limit`
    waits of each instruction onto same-engine no-ops inserted right before
    it.  Engine-level serialization keeps the semantics identical."""
    k = 0
    for f in nc.m.functions:
        for b in f.blocks:
            out = []
            changed = False
            for inst in b.instructions:
                si = inst.sync_info
                if si is not None and si.on_wait and len(si.on_wait) > limit:
                    waits = list(si.on_wait)
                    for w in waits[:-limit]:
                        nop = mybir.InstNoOp(name=f"I-nw{k}", ins=[], outs=[])
                        k += 1
                        nop.engine = inst.engine
                        nop.sync_info = mybir.SyncInfo(on_wait=[w], on_update=[])
                        out.append(nop)
                    inst.sync_info = mybir.SyncInfo(
                        on_wait=waits[-limit:], on_update=list(si.on_update))
                    changed = True
                out.append(inst)
            if changed:
                b.instructions = out


# ---------------------------------------------------------------- program
def build_program(n_tokens=TOK_PER_CORE, split_waits=True):
    from contextlib import ExitStack
    nc = bass.Bass("TRN2", target_bir_lowering=False, debug=False,
                   num_devices=N_CORES)
    ins = {
        "x": nc.dram_tensor("x", [n_tokens, D], F32, kind="ExternalInput").ap(),
        "w1": nc.dram_tensor("w1", [4 * SK, D], F32, kind="ExternalInput").ap(),
        "w2": nc.dram_tensor("w2", [4 * SK, D], F32, kind="ExternalInput").ap(),
        "hsel1": nc.dram_tensor("hsel1", [128, 128], F32, kind="ExternalInput").ap(),
        "hsel2": nc.dram_tensor("hsel2", [128, 128], F32, kind="ExternalInput").ap(),
        "ident": nc.dram_tensor("ident", [128, 128], F32, kind="ExternalInput").ap(),
    }
    outs = {
        "scores": nc.dram_tensor("scores", [n_tokens, KOUT], F32,
                                 kind="ExternalOutput").ap(),
        "idx": nc.dram_tensor("idx", [n_tokens, TOPK], I32,
                              kind="ExternalOutput").ap(),
        "gates": nc.dram_tensor("gates", [n_tokens, TOPK], F32,
                                kind="ExternalOutput").ap(),
    }
    with tile.TileContext(nc) as tc:
        with ExitStack() as ctx:
            router_tile_kernel(tc, outs, ins, n_tokens, ctx)
    if split_waits:
        _split_multi_waits(nc)
    return nc


_CACHED = {}
LAST_RESULTS = None


def kernel(x, W1, W2):
    """Full-input entry point: shards tokens over 8 cores, returns full outputs."""
    global LAST_RESULTS
    x = np.ascontiguousarray(np.asarray(x, dtype=np.float32))
    W1 = np.ascontiguousarray(np.asarray(W1, dtype=np.float32))
    W2 = np.ascontiguousarray(np.asarray(W2, dtype=np.float32))
    B, S, _ = x.shape
    xf = x.reshape(B * S, D)
    assert B * S == TOK_TOTAL

    if "nc" not in _CACHED:
        _CACHED["nc"] = build_program(TOK_PER_CORE)
    nc = _CACHED["nc"]

    consts = make_consts()
    in_maps = []
    for c in range(N_CORES):
        shard = np.ascontiguousarray(
            xf[c * TOK_PER_CORE:(c + 1) * TOK_PER_CORE])
        in_maps.append({"x": shard, "w1": W1, "w2": W2, **consts})

    trace = bool(int(os.environ.get("ROUTER_TRACE", "0")))
    res = bass_utils.run_bass_kernel_spmd(
        nc, in_maps, core_ids=list(range(N_CORES)), trace=trace)
    LAST_RESULTS = res

    scores = np.concatenate([r["scores"] for r in res.results], axis=0)
    idx = np.concatenate([r["idx"] for r in res.results], axis=0)
    gates = np.concatenate([r["gates"] for r in res.results], axis=0)
    return (idx.reshape(B, S, TOPK).astype(np.int32),
            gates.reshape(B, S, TOPK),
            scores.reshape(B, S, KOUT))


# revision 25
# speedup vs baseline: 1.2473x; 1.2473x over previous
"""MultiHeadProductKeyRouter Trainium2 Bass kernel.

Math: reference computes
    s1 = x @ W1.T  -> (T, H*SK) -> (T, H, SK)
    s2 = x @ W2.T
    head_scores[t,h,i*SK+j] = s1[t,h,i] + s2[t,h,j]
    scores = head_scores.mean(axis=-2)          # (T, SK*SK)
    topk_vals, topk_idx = top_k(scores, 8); gates = softmax(topk_vals)

Since the mean over heads commutes with the outer-sum:
    scores[t, i*SK+j] = a1[t,i] + a2[t,j]
with a1 = mean_h s1, a2 = mean_h s2, i.e. a = x @ Vall.T where
Vall = [mean_h W1_heads ; mean_h W2_heads]  (128 x 2048).

scores is produced on the TensorEngine as A @ M where A = [a1|a2] (128
features per token) and M is a constant 128x4096 0/1 matrix with
M[i, i*64+j] = 1 and M[64+j, i*64+j] = 1.  Each PSUM element is then a
single fp32 add a1[i]+a2[j] (zeros accumulate exactly), bit-identical to
the candidate sums used for top-k.

Top-8 of the outer sum: any top-8 element must have i in top8(a1) and
j in top8(a2), so top-8 over the 64 candidate sums of top8(a1) x top8(a2)
equals top-8 of the full 4096 row.  DVE max/max_index provide top-8
values+indices directly; candidate position is decoded as (p,q) = (c//8,
c%8) and mapped through the top-8 index lists.

Sharding: data parallel over tokens, 16384 tokens -> 8 cores x 2048.
"""

import os
import numpy as np

import concourse.bass as bass
import concourse.mybir as mybir
import concourse.tile as tile
from concourse import bass_utils

F32 = mybir.dt.float32
U32 = mybir.dt.uint32
I32 = mybir.dt.int32
Alu = mybir.AluOpType

D = 2048          # model dim
SK = 64           # sqrt(num experts)
KOUT = SK * SK    # 4096
NF = 128          # features after head-mean: 64 (a1) + 64 (a2)
TOPK = 8
DC = D // 128     # 16 contraction chunks
N_CORES = 8
TOK_TOTAL = 4 * 4096
TOK_PER_CORE = TOK_TOTAL // N_CORES   # 2048
GROUP = 512       # tokens per matmul1 group
SUB = GROUP // 128


# ---------------------------------------------------------------- host consts
def make_consts():
    # Head-sum selectors: W tiles are [128 rows = 2 heads x 64] ; out column
    # (i or 64+j) accumulates 0.25 * row (p % 64).
    hsel1 = np.zeros((128, 128), np.float32)
    hsel2 = np.zeros((128, 128), np.float32)
    for p in range(128):
        hsel1[p, p % SK] = 0.25
        hsel2[p, SK + (p % SK)] = 0.25
    ident = np.eye(128, dtype=np.float32)
    return {"hsel1": hsel1, "hsel2": hsel2, "ident": ident}


# Number of trailing i-blocks (of 8 rows of 64 scores each) of the outer-sum
# handed to GpSimd per subtile; rest goes to DVE.
GPSIMD_IBLOCKS = 0


def _bc_outer(a, m):
    """[128, n] -> [128, n, m] view repeating each element (stride-0 inner)."""
    return bass.AP(tensor=a.tensor, offset=a.offset,
                   ap=[a.ap[0], a.ap[1], [0, m]])


def _bc_inner(a, n):
    """[128, m] -> [128, n, m] view repeating the row n times (stride-0 outer)."""
    return bass.AP(tensor=a.tensor, offset=a.offset,
                   ap=[a.ap[0], [0, n], a.ap[1]])


def _r3(sc, m):
    return sc.rearrange("p (i j) -> p i j", j=m)


# ---------------------------------------------------------------- tile kernel
def router_tile_kernel(tc, outs, ins, n_tokens, ctx):
    nc = tc.nc
    n_groups = n_tokens // GROUP

    x = ins["x"]            # [n_tokens, D]
    w1 = ins["w1"]          # [256, D]
    w2 = ins["w2"]          # [256, D]
    hsel1 = ins["hsel1"]    # [128, 128]
    hsel2 = ins["hsel2"]    # [128, 128]
    ident = ins["ident"]    # [128, 128]
    scores_d = outs["scores"]   # [n_tokens, KOUT] f32
    idx_d = outs["idx"]         # [n_tokens, TOPK] i32
    gates_d = outs["gates"]     # [n_tokens, TOPK] f32

    consts = ctx.enter_context(tc.tile_pool(name="consts", bufs=1))
    wpool = ctx.enter_context(tc.tile_pool(name="wpool", bufs=2))
    xpool = ctx.enter_context(tc.tile_pool(name="xpool", bufs=2))
    xtpool = ctx.enter_context(tc.tile_pool(name="xtpool", bufs=2))
    apool = ctx.enter_context(tc.tile_pool(name="apool", bufs=2))
    atpool = ctx.enter_context(tc.tile_pool(name="atpool", bufs=2))
    spool = ctx.enter_context(tc.tile_pool(name="spool", bufs=3))
    small = ctx.enter_context(tc.tile_pool(name="small", bufs=3))
    ps_tr = ctx.enter_context(tc.tile_pool(name="ps_tr", bufs=4, space="PSUM"))
    ps_mm1 = ctx.enter_context(tc.tile_pool(name="ps_mm1", bufs=2, space="PSUM"))
    ps_at = ctx.enter_context(tc.tile_pool(name="ps_at", bufs=2, space="PSUM"))

    # ---- constants into SBUF
    hs1_sb = consts.tile([128, 128], F32)
    nc.sync.dma_start(hs1_sb, hsel1)
    hs2_sb = consts.tile([128, 128], F32)
    nc.sync.dma_start(hs2_sb, hsel2)
    id_sb = consts.tile([128, 128], F32)
    nc.sync.dma_start(id_sb, ident)

    # ---- Vall = 0.25 * head-sum of [W1;W2]  -> [128 f, D]
    vall_sb = consts.tile([128, D], F32)
    pv = [ps_tr.tile([128, 512], F32, tag="tr", name=f"pv{c4}")
          for c4 in range(D // 512)]
    for i, (w, hs) in enumerate(
            ((w1, hs1_sb), (w1, hs1_sb), (w2, hs2_sb), (w2, hs2_sb))):
        r = i % 2
        wt = wpool.tile([128, D], F32, tag="w")
        nc.sync.dma_start(wt, w[r * 128:(r + 1) * 128, :])
        for c4 in range(D // 512):
            nc.tensor.matmul(pv[c4], hs, wt[:, c4 * 512:(c4 + 1) * 512],
                             start=(i == 0), stop=(i == 3))
    for c4 in range(D // 512):
        nc.vector.tensor_copy(vall_sb[:, c4 * 512:(c4 + 1) * 512], pv[c4])

    # ---- VallT chunks [128 d, 128 f] for matmul1 stationary
    vallT_sb = consts.tile([128, DC, 128], F32)
    for c4 in range(DC // 4):
        pt = ps_tr.tile([128, 512], F32, tag="tr")
        for cc in range(4):
            c = c4 * 4 + cc
            nc.tensor.transpose(pt[:, cc * 128:(cc + 1) * 128],
                                vall_sb[:, c * 128:(c + 1) * 128], id_sb)
        nc.vector.tensor_copy(vallT_sb[:, c4 * 4:(c4 + 1) * 4, :], pt)

    # ---- main loop over token groups
    for g in range(n_groups):
        xt = xtpool.tile([128, DC, GROUP], F32, tag="xt")  # [d-chunk part, c, t]
        for s in range(SUB):
            xs = xpool.tile([128, D], F32, tag="x")
            nc.sync.dma_start(xs, x[(g * SUB + s) * 128:(g * SUB + s + 1) * 128, :])
            for c4 in range(DC // 4):
                pt = ps_tr.tile([128, 512], F32, tag="tr")
                for cc in range(4):
                    c = c4 * 4 + cc
                    nc.tensor.transpose(pt[:, cc * 128:(cc + 1) * 128],
                                        xs[:, c * 128:(c + 1) * 128], id_sb)
                # one strided copy: psum [128,4,128] -> xt[:, 4c4:4c4+4, s*128:+128]
                dst = xt[:, c4 * 4:(c4 + 1) * 4, s * 128:(s + 1) * 128]
                nc.scalar.copy(dst, pt.rearrange("p (c t) -> p c t", c=4))

        # matmul1: A[f, t] = sum_c VallT[c].T @ xt[c]
        a_ps = ps_mm1.tile([128, GROUP], F32, tag="mm1")
        for c in range(DC):
            nc.tensor.matmul(a_ps, vallT_sb[:, c, :], xt[:, c, :],
                             start=(c == 0), stop=(c == DC - 1))
        a_sb = apool.tile([128, GROUP], F32, tag="a")
        nc.scalar.copy(a_sb, a_ps)

        # At[t, f] per subtile (for top-k)
        at_ps = ps_at.tile([128, SUB, 128], F32, tag="at")
        for s in range(SUB):
            nc.tensor.transpose(at_ps[:, s, :], a_sb[:, s * 128:(s + 1) * 128], id_sb)
        at_sb = atpool.tile([128, SUB, 128], F32, tag="at")
        nc.scalar.copy(at_sb, at_ps)

        # scores outer-sum + topk per subtile
        for s in range(SUB):
            tok0 = (g * SUB + s) * 128
            at = at_sb[:, s, :]
            a1 = at[:, 0:SK]
            a2 = at[:, SK:2 * SK]
            sc_sb = spool.tile([128, KOUT], F32, tag="sc")
            # scores[t, i*64+j] = a1[t,i] + a2[t,j] via stride-0 broadcast APs
            if GPSIMD_IBLOCKS:
                split = SK - GPSIMD_IBLOCKS * 8
                nc.vector.tensor_add(
                    _r3(sc_sb[:, :split * SK], SK),
                    _bc_outer(a1[:, :split], SK), _bc_inner(a2, split))
                nc.gpsimd.tensor_add(
                    _r3(sc_sb[:, split * SK:], SK),
                    _bc_outer(a1[:, split:], SK), _bc_inner(a2, SK - split))
            else:
                nc.vector.tensor_add(
                    _r3(sc_sb, SK), _bc_outer(a1, SK), _bc_inner(a2, SK))
            nc.sync.dma_start(scores_d[tok0:tok0 + 128, :], sc_sb)

            _topk_block(nc, small, at, idx_d, gates_d, tok0)


def _topk_block(nc, small, at, idx_d, gates_d, tok0):
    """Top-8 + gates for one 128-token subtile. at: [128 t, 128 f] SBUF."""
    a1 = at[:, 0:SK]
    a2 = at[:, SK:2 * SK]
    t8 = [128, TOPK]

    v1 = small.tile(t8, F32, tag="v1")
    v2 = small.tile(t8, F32, tag="v2")
    i1u = small.tile(t8, U32, tag="i1u")
    i2u = small.tile(t8, U32, tag="i2u")
    i1f = small.tile(t8, F32, tag="i1f")
    i2f = small.tile(t8, F32, tag="i2f")
    i1n = small.tile(t8, F32, tag="i1n")
    cv = small.tile([128, 64], F32, tag="cv")
    cin = small.tile([128, 64], F32, tag="cin")
    tv = small.tile(t8, F32, tag="tv")
    eqm = small.tile([128, TOPK, 64], F32, tag="eqm")
    red = small.tile(t8, F32, tag="red")
    flatf = small.tile(t8, F32, tag="flatf")
    idx_sb = small.tile(t8, I32, tag="idx_sb")
    negm = small.tile([128, 1], F32, tag="negm")
    e8 = small.tile(t8, F32, tag="e8")
    ssum = small.tile([128, 1], F32, tag="ssum")
    rinv = small.tile([128, 1], F32, tag="rinv")
    gates_sb = small.tile(t8, F32, tag="gates_sb")

    # top-8 of each half (values sorted desc; indices = first occurrence)
    nc.vector.max(v1, a1)
    nc.vector.max_index(i1u, v1, a1)
    nc.vector.max(v2, a2)
    nc.vector.max_index(i2u, v2, a2)
    nc.vector.tensor_copy(i1f, i1u)
    nc.vector.tensor_copy(i2f, i2u)

    # 64 candidates cv[p*8+q] = v1[p] + v2[q]; the exact same fp32 adds as
    # the scores outer-sum, so values are bit-identical to scores entries.
    nc.vector.tensor_add(_r3(cv, 8), _bc_outer(v1, 8), _bc_inner(v2, 8))
    # negated flat index per candidate: cin[p*8+q] = 4096 - (64*i1[p] + i2[q])
    nc.vector.tensor_scalar(i1n, i1f, -64.0, 4096.0, op0=Alu.mult, op1=Alu.add)
    nc.vector.tensor_tensor(_r3(cin, 8), _bc_outer(i1n, 8), _bc_inner(i2f, 8),
                            op=Alu.subtract)

    nc.vector.max(tv, cv)

    # flat idx of the k-th winner: match tv[k] against cv, pick the matching
    # candidate's min flat idx (max of cin), all in one masked reduce.
    nc.vector.tensor_tensor(eqm, _bc_inner(cv, 8),
                            _bc_outer(tv, 64), op=Alu.is_equal)
    nc.vector.tensor_mul(eqm, eqm, _bc_inner(cin, 8))
    nc.vector.reduce_max(red, eqm, axis=mybir.AxisListType.X)
    nc.vector.tensor_scalar(flatf, red, -1.0, 4096.0, op0=Alu.mult, op1=Alu.add)
    nc.vector.tensor_copy(idx_sb, flatf)

    # gates = softmax(tv) (tv[:,0] is the max)
    nc.vector.tensor_scalar_mul(negm, tv[:, 0:1], -1.0)
    nc.scalar.activation(e8, tv, mybir.ActivationFunctionType.Exp,
                         bias=negm, scale=1.0, accum_out=ssum)
    nc.vector.reciprocal(rinv, ssum)
    nc.vector.tensor_scalar_mul(gates_sb, e8, rinv)

    nc.sync.dma_start(idx_d[tok0:tok0 + 128, :], idx_sb)
    nc.sync.dma_start(gates_d[tok0:tok0 + 128, :], gates_sb)


def _split_multi_waits(nc, limit=1):
    """Walrus in this toolchain rejects instructions carrying more than one
    semaphore wait (fp32 Matmult LDW path asserts at even 2, end-of-kernel
    drains at 5).  Post-process the scheduled BIR: move all but `limit`
    waits of each instruction onto same-engine no-ops inserted right before
    it.  Engine-level serialization keeps the semantics identical."""
    k = 0
    for f in nc.m.functions:
        for b in f.blocks:
            out = []
            changed = False
            for inst in b.instructions:
                si = inst.sync_info
                if si is not None and si.on_wait and len(si.on_wait) > limit:
                    waits = list(si.on_wait)
                    for w in waits[:-limit]:
                        nop = mybir.InstNoOp(name=f"I-nw{k}", ins=[], outs=[])
                        k += 1
                        nop.engine = inst.engine
                        nop.sync_info = mybir.SyncInfo(on_wait=[w], on_update=[])
                        out.append(nop)
                    inst.sync_info = mybir.SyncInfo(
                        on_wait=waits[-limit:], on_update=list(si.on_update))
                    changed = True
                out.append(inst)
            if changed:
                b.instructions = out


# ---------------------------------------------------------------- program
def build_program(n_tokens=TOK_PER_CORE, split_waits=True):
    from contextlib import ExitStack
    nc = bass.Bass("TRN2", target_bir_lowering=False, debug=False,
                   num_devices=N_CORES)
    ins = {
        "x": nc.dram_tensor("x", [n_tokens, D], F32, kind="ExternalInput").ap(),
        "w1": nc.dram_tensor("w1", [4 * SK, D], F32, kind="ExternalInput").ap(),
        "w2": nc.dram_tensor("w2", [4 * SK, D], F32, kind="ExternalInput").ap(),
        "hsel1": nc.dram_tensor("hsel1", [128, 128], F32, kind="ExternalInput").ap(),
        "hsel2": nc.dram_tensor("hsel2", [128, 128], F32, kind="ExternalInput").ap(),
        "ident": nc.dram_tensor("ident", [128, 128], F32, kind="ExternalInput").ap(),
    }
    outs = {
        "scores": nc.dram_tensor("scores", [n_tokens, KOUT], F32,
                                 kind="ExternalOutput").ap(),
        "idx": nc.dram_tensor("idx", [n_tokens, TOPK], I32,
                              kind="ExternalOutput").ap(),
        "gates": nc.dram_tensor("gates", [n_tokens, TOPK], F32,
                                kind="ExternalOutput").ap(),
    }
    with tile.TileContext(nc) as tc:
        with ExitStack() as ctx:
            router_tile_kernel(tc, outs, ins, n_tokens, ctx)
    if split_waits:
        _split_multi_waits(nc)
    return nc


_CACHED = {}
LAST_RESULTS = None


def kernel(x, W1, W2):
    """Full-input entry point: shards tokens over 8 cores, returns full outputs."""
    global LAST_RESULTS
    x = np.ascontiguousarray(np.asarray(x, dtype=np.float32))
    W1 = np.ascontiguousarray(np.asarray(W1, dtype=np.float32))
    W2 = np.ascontiguousarray(np.asarray(W2, dtype=np.float32))
    B, S, _ = x.shape
    xf = x.reshape(B * S, D)
    assert B * S == TOK_TOTAL

    if "nc" not in _CACHED:
        _CACHED["nc"] = build_program(TOK_PER_CORE)
    nc = _CACHED["nc"]

    consts = make_consts()
    in_maps = []
    for c in range(N_CORES):
        shard = np.ascontiguousarray(
            xf[c * TOK_PER_CORE:(c + 1) * TOK_PER_CORE])
        in_maps.append({"x": shard, "w1": W1, "w2": W2, **consts})

    trace = bool(int(os.environ.get("ROUTER_TRACE", "0")))
    res = bass_utils.run_bass_kernel_spmd(
        nc, in_maps, core_ids=list(range(N_CORES)), trace=trace)
    LAST_RESULTS = res

    scores = np.concatenate([r["scores"] for r in res.results], axis=0)
    idx = np.concatenate([r["idx"] for r in res.results], axis=0)
    gates = np.concatenate([r["gates"] for r in res.results], axis=0)
    return (idx.reshape(B, S, TOPK).astype(np.int32),
            gates.reshape(B, S, TOPK),
            scores.reshape(B, S, KOUT))


# revision 26
# speedup vs baseline: 1.2747x; 1.0220x over previous
"""MultiHeadProductKeyRouter Trainium2 Bass kernel.

Math: reference computes
    s1 = x @ W1.T  -> (T, H*SK) -> (T, H, SK)
    s2 = x @ W2.T
    head_scores[t,h,i*SK+j] = s1[t,h,i] + s2[t,h,j]
    scores = head_scores.mean(axis=-2)          # (T, SK*SK)
    topk_vals, topk_idx = top_k(scores, 8); gates = softmax(topk_vals)

Since the mean over heads commutes with the outer-sum:
    scores[t, i*SK+j] = a1[t,i] + a2[t,j]
with a1 = mean_h s1, a2 = mean_h s2, i.e. a = x @ Vall.T where
Vall = [mean_h W1_heads ; mean_h W2_heads]  (128 x 2048).

scores is produced on the TensorEngine as A @ M where A = [a1|a2] (128
features per token) and M is a constant 128x4096 0/1 matrix with
M[i, i*64+j] = 1 and M[64+j, i*64+j] = 1.  Each PSUM element is then a
single fp32 add a1[i]+a2[j] (zeros accumulate exactly), bit-identical to
the candidate sums used for top-k.

Top-8 of the outer sum: any top-8 element must have i in top8(a1) and
j in top8(a2), so top-8 over the 64 candidate sums of top8(a1) x top8(a2)
equals top-8 of the full 4096 row.  DVE max/max_index provide top-8
values+indices directly; candidate position is decoded as (p,q) = (c//8,
c%8) and mapped through the top-8 index lists.

Sharding: data parallel over tokens, 16384 tokens -> 8 cores x 2048.
"""

import os
import numpy as np

import concourse.bass as bass
import concourse.mybir as mybir
import concourse.tile as tile
from concourse import bass_utils

F32 = mybir.dt.float32
U32 = mybir.dt.uint32
I32 = mybir.dt.int32
Alu = mybir.AluOpType

D = 2048          # model dim
SK = 64           # sqrt(num experts)
KOUT = SK * SK    # 4096
NF = 128          # features after head-mean: 64 (a1) + 64 (a2)
TOPK = 8
DC = D // 128     # 16 contraction chunks
N_CORES = 8
TOK_TOTAL = 4 * 4096
TOK_PER_CORE = TOK_TOTAL // N_CORES   # 2048
GROUP = 512       # tokens per matmul1 group
SUB = GROUP // 128


# ---------------------------------------------------------------- host consts
def make_consts():
    # Head-sum selectors: W tiles are [128 rows = 2 heads x 64] ; out column
    # (i or 64+j) accumulates 0.25 * row (p % 64).
    hsel1 = np.zeros((128, 128), np.float32)
    hsel2 = np.zeros((128, 128), np.float32)
    for p in range(128):
        hsel1[p, p % SK] = 0.25
        hsel2[p, SK + (p % SK)] = 0.25
    ident = np.eye(128, dtype=np.float32)
    return {"hsel1": hsel1, "hsel2": hsel2, "ident": ident}


# Number of trailing i-blocks (of 8 rows of 64 scores each) of the outer-sum
# handed to GpSimd per subtile; rest goes to DVE.
GPSIMD_IBLOCKS = 0


def _bc_outer(a, m):
    """[128, n] -> [128, n, m] view repeating each element (stride-0 inner)."""
    return bass.AP(tensor=a.tensor, offset=a.offset,
                   ap=[a.ap[0], a.ap[1], [0, m]])


def _bc_inner(a, n):
    """[128, m] -> [128, n, m] view repeating the row n times (stride-0 outer)."""
    return bass.AP(tensor=a.tensor, offset=a.offset,
                   ap=[a.ap[0], [0, n], a.ap[1]])


def _r3(sc, m):
    return sc.rearrange("p (i j) -> p i j", j=m)


# ---------------------------------------------------------------- tile kernel
def router_tile_kernel(tc, outs, ins, n_tokens, ctx):
    nc = tc.nc
    n_groups = n_tokens // GROUP

    x = ins["x"]            # [n_tokens, D]
    w1 = ins["w1"]          # [256, D]
    w2 = ins["w2"]          # [256, D]
    hsel1 = ins["hsel1"]    # [128, 128]
    hsel2 = ins["hsel2"]    # [128, 128]
    ident = ins["ident"]    # [128, 128]
    scores_d = outs["scores"]   # [n_tokens, KOUT] f32
    idx_d = outs["idx"]         # [n_tokens, TOPK] i32
    gates_d = outs["gates"]     # [n_tokens, TOPK] f32

    consts = ctx.enter_context(tc.tile_pool(name="consts", bufs=1))
    wpool = ctx.enter_context(tc.tile_pool(name="wpool", bufs=2))
    xpool = ctx.enter_context(tc.tile_pool(name="xpool", bufs=2))
    xtpool = ctx.enter_context(tc.tile_pool(name="xtpool", bufs=2))
    apool = ctx.enter_context(tc.tile_pool(name="apool", bufs=2))
    atpool = ctx.enter_context(tc.tile_pool(name="atpool", bufs=2))
    spool = ctx.enter_context(tc.tile_pool(name="spool", bufs=3))
    small = ctx.enter_context(tc.tile_pool(name="small", bufs=3))
    ps_tr = ctx.enter_context(tc.tile_pool(name="ps_tr", bufs=4, space="PSUM"))
    ps_mm1 = ctx.enter_context(tc.tile_pool(name="ps_mm1", bufs=2, space="PSUM"))
    ps_at = ctx.enter_context(tc.tile_pool(name="ps_at", bufs=2, space="PSUM"))

    # ---- constants into SBUF
    hs1_sb = consts.tile([128, 128], F32)
    nc.sync.dma_start(hs1_sb, hsel1)
    hs2_sb = consts.tile([128, 128], F32)
    nc.sync.dma_start(hs2_sb, hsel2)
    id_sb = consts.tile([128, 128], F32)
    nc.sync.dma_start(id_sb, ident)

    # ---- Vall = 0.25 * head-sum of [W1;W2]  -> [128 f, D]
    vall_sb = consts.tile([128, D], F32)
    pv = [ps_tr.tile([128, 512], F32, tag="tr", name=f"pv{c4}")
          for c4 in range(D // 512)]
    for i, (w, hs) in enumerate(
            ((w1, hs1_sb), (w1, hs1_sb), (w2, hs2_sb), (w2, hs2_sb))):
        r = i % 2
        wt = wpool.tile([128, D], F32, tag="w")
        nc.sync.dma_start(wt, w[r * 128:(r + 1) * 128, :])
        for c4 in range(D // 512):
            nc.tensor.matmul(pv[c4], hs, wt[:, c4 * 512:(c4 + 1) * 512],
                             start=(i == 0), stop=(i == 3))
    for c4 in range(D // 512):
        nc.vector.tensor_copy(vall_sb[:, c4 * 512:(c4 + 1) * 512], pv[c4])

    # ---- VallT chunks [128 d, 128 f] for matmul1 stationary
    vallT_sb = consts.tile([128, DC, 128], F32)
    for c4 in range(DC // 4):
        pt = ps_tr.tile([128, 512], F32, tag="tr")
        for cc in range(4):
            c = c4 * 4 + cc
            nc.tensor.transpose(pt[:, cc * 128:(cc + 1) * 128],
                                vall_sb[:, c * 128:(c + 1) * 128], id_sb)
        nc.vector.tensor_copy(vallT_sb[:, c4 * 4:(c4 + 1) * 4, :], pt)

    # ---- main loop over token groups
    for g in range(n_groups):
        xt = xtpool.tile([128, DC, GROUP], F32, tag="xt")  # [d-chunk part, c, t]
        for s in range(SUB):
            xs = xpool.tile([128, D], F32, tag="x")
            # x loads ride the ACT HWDGE ring so they are not FIFO-ordered
            # behind the large score stores on the SP ring
            nc.scalar.dma_start(xs, x[(g * SUB + s) * 128:(g * SUB + s + 1) * 128, :])
            for c4 in range(DC // 4):
                pt = ps_tr.tile([128, 512], F32, tag="tr")
                for cc in range(4):
                    c = c4 * 4 + cc
                    nc.tensor.transpose(pt[:, cc * 128:(cc + 1) * 128],
                                        xs[:, c * 128:(c + 1) * 128], id_sb)
                # one strided copy: psum [128,4,128] -> xt[:, 4c4:4c4+4, s*128:+128]
                dst = xt[:, c4 * 4:(c4 + 1) * 4, s * 128:(s + 1) * 128]
                nc.scalar.copy(dst, pt.rearrange("p (c t) -> p c t", c=4))

        # matmul1: A[f, t] = sum_c VallT[c].T @ xt[c]
        a_ps = ps_mm1.tile([128, GROUP], F32, tag="mm1")
        for c in range(DC):
            nc.tensor.matmul(a_ps, vallT_sb[:, c, :], xt[:, c, :],
                             start=(c == 0), stop=(c == DC - 1))
        a_sb = apool.tile([128, GROUP], F32, tag="a")
        nc.scalar.copy(a_sb, a_ps)

        # At[t, f] per subtile (for top-k)
        at_ps = ps_at.tile([128, SUB, 128], F32, tag="at")
        for s in range(SUB):
            nc.tensor.transpose(at_ps[:, s, :], a_sb[:, s * 128:(s + 1) * 128], id_sb)
        at_sb = atpool.tile([128, SUB, 128], F32, tag="at")
        nc.scalar.copy(at_sb, at_ps)

        # scores outer-sum + topk per subtile
        for s in range(SUB):
            tok0 = (g * SUB + s) * 128
            at = at_sb[:, s, :]
            a1 = at[:, 0:SK]
            a2 = at[:, SK:2 * SK]
            sc_sb = spool.tile([128, KOUT], F32, tag="sc")
            # scores[t, i*64+j] = a1[t,i] + a2[t,j] via stride-0 broadcast APs
            if GPSIMD_IBLOCKS:
                split = SK - GPSIMD_IBLOCKS * 8
                nc.vector.tensor_add(
                    _r3(sc_sb[:, :split * SK], SK),
                    _bc_outer(a1[:, :split], SK), _bc_inner(a2, split))
                nc.gpsimd.tensor_add(
                    _r3(sc_sb[:, split * SK:], SK),
                    _bc_outer(a1[:, split:], SK), _bc_inner(a2, SK - split))
            else:
                nc.vector.tensor_add(
                    _r3(sc_sb, SK), _bc_outer(a1, SK), _bc_inner(a2, SK))
            nc.sync.dma_start(scores_d[tok0:tok0 + 128, :], sc_sb)

            _topk_block(nc, small, at, idx_d, gates_d, tok0)


def _topk_block(nc, small, at, idx_d, gates_d, tok0):
    """Top-8 + gates for one 128-token subtile. at: [128 t, 128 f] SBUF."""
    a1 = at[:, 0:SK]
    a2 = at[:, SK:2 * SK]
    t8 = [128, TOPK]

    v1 = small.tile(t8, F32, tag="v1")
    v2 = small.tile(t8, F32, tag="v2")
    i1u = small.tile(t8, U32, tag="i1u")
    i2u = small.tile(t8, U32, tag="i2u")
    i1f = small.tile(t8, F32, tag="i1f")
    i2f = small.tile(t8, F32, tag="i2f")
    i1n = small.tile(t8, F32, tag="i1n")
    cv = small.tile([128, 64], F32, tag="cv")
    cin = small.tile([128, 64], F32, tag="cin")
    tv = small.tile(t8, F32, tag="tv")
    eqm = small.tile([128, TOPK, 64], F32, tag="eqm")
    red = small.tile(t8, F32, tag="red")
    flatf = small.tile(t8, F32, tag="flatf")
    idx_sb = small.tile(t8, I32, tag="idx_sb")
    negm = small.tile([128, 1], F32, tag="negm")
    e8 = small.tile(t8, F32, tag="e8")
    ssum = small.tile([128, 1], F32, tag="ssum")
    rinv = small.tile([128, 1], F32, tag="rinv")
    gates_sb = small.tile(t8, F32, tag="gates_sb")

    # top-8 of each half (values sorted desc; indices = first occurrence)
    nc.vector.max(v1, a1)
    nc.vector.max_index(i1u, v1, a1)
    nc.vector.max(v2, a2)
    nc.vector.max_index(i2u, v2, a2)
    nc.vector.tensor_copy(i1f, i1u)
    nc.vector.tensor_copy(i2f, i2u)

    # 64 candidates cv[p*8+q] = v1[p] + v2[q]; the exact same fp32 adds as
    # the scores outer-sum, so values are bit-identical to scores entries.
    nc.vector.tensor_add(_r3(cv, 8), _bc_outer(v1, 8), _bc_inner(v2, 8))
    # negated flat index per candidate: cin[p*8+q] = 4096 - (64*i1[p] + i2[q])
    nc.vector.tensor_scalar(i1n, i1f, -64.0, 4096.0, op0=Alu.mult, op1=Alu.add)
    nc.vector.tensor_tensor(_r3(cin, 8), _bc_outer(i1n, 8), _bc_inner(i2f, 8),
                            op=Alu.subtract)

    nc.vector.max(tv, cv)

    # flat idx of the k-th winner: match tv[k] against cv, pick the matching
    # candidate's min flat idx (max of cin), all in one masked reduce.
    nc.vector.tensor_tensor(eqm, _bc_inner(cv, 8),
                            _bc_outer(tv, 64), op=Alu.is_equal)
    nc.vector.tensor_mul(eqm, eqm, _bc_inner(cin, 8))
    nc.vector.reduce_max(red, eqm, axis=mybir.AxisListType.X)
    nc.vector.tensor_scalar(flatf, red, -1.0, 4096.0, op0=Alu.mult, op1=Alu.add)
    nc.vector.tensor_copy(idx_sb, flatf)

    # gates = softmax(tv) (tv[:,0] is the max)
    nc.vector.tensor_scalar_mul(negm, tv[:, 0:1], -1.0)
    nc.scalar.activation(e8, tv, mybir.ActivationFunctionType.Exp,
                         bias=negm, scale=1.0, accum_out=ssum)
    nc.vector.reciprocal(rinv, ssum)
    nc.vector.tensor_scalar_mul(gates_sb, e8, rinv)

    nc.sync.dma_start(idx_d[tok0:tok0 + 128, :], idx_sb)
    nc.sync.dma_start(gates_d[tok0:tok0 + 128, :], gates_sb)


def _split_multi_waits(nc, limit=1):
    """Walrus in this toolchain rejects instructions carrying more than one
    semaphore wait (fp32 Matmult LDW path asserts at even 2, end-of-kernel
    drains at 5).  Post-process the scheduled BIR: move all but `limit`
    waits of each instruction onto same-engine no-ops inserted right before
    it.  Engine-level serialization keeps the semantics identical."""
    k = 0
    for f in nc.m.functions:
        for b in f.blocks:
            out = []
            changed = False
            for inst in b.instructions:
                si = inst.sync_info
                if si is not None and si.on_wait and len(si.on_wait) > limit:
                    waits = list(si.on_wait)
                    for w in waits[:-limit]:
                        nop = mybir.InstNoOp(name=f"I-nw{k}", ins=[], outs=[])
                        k += 1
                        nop.engine = inst.engine
                        nop.sync_info = mybir.SyncInfo(on_wait=[w], on_update=[])
                        out.append(nop)
                    inst.sync_info = mybir.SyncInfo(
                        on_wait=waits[-limit:], on_update=list(si.on_update))
                    changed = True
                out.append(inst)
            if changed:
                b.instructions = out


# ---------------------------------------------------------------- program
def build_program(n_tokens=TOK_PER_CORE, split_waits=True):
    from contextlib import ExitStack
    nc = bass.Bass("TRN2", target_bir_lowering=False, debug=False,
                   num_devices=N_CORES)
    ins = {
        "x": nc.dram_tensor("x", [n_tokens, D], F32, kind="ExternalInput").ap(),
        "w1": nc.dram_tensor("w1", [4 * SK, D], F32, kind="ExternalInput").ap(),
        "w2": nc.dram_tensor("w2", [4 * SK, D], F32, kind="ExternalInput").ap(),
        "hsel1": nc.dram_tensor("hsel1", [128, 128], F32, kind="ExternalInput").ap(),
        "hsel2": nc.dram_tensor("hsel2", [128, 128], F32, kind="ExternalInput").ap(),
        "ident": nc.dram_tensor("ident", [128, 128], F32, kind="ExternalInput").ap(),
    }
    outs = {
        "scores": nc.dram_tensor("scores", [n_tokens, KOUT], F32,
                                 kind="ExternalOutput").ap(),
        "idx": nc.dram_tensor("idx", [n_tokens, TOPK], I32,
                              kind="ExternalOutput").ap(),
        "gates": nc.dram_tensor("gates", [n_tokens, TOPK], F32,
                                kind="ExternalOutput").ap(),
    }
    with tile.TileContext(nc) as tc:
        with ExitStack() as ctx:
            router_tile_kernel(tc, outs, ins, n_tokens, ctx)
    if split_waits:
        _split_multi_waits(nc)
    return nc


_CACHED = {}
LAST_RESULTS = None


def kernel(x, W1, W2):
    """Full-input entry point: shards tokens over 8 cores, returns full outputs."""
    global LAST_RESULTS
    x = np.ascontiguousarray(np.asarray(x, dtype=np.float32))
    W1 = np.ascontiguousarray(np.asarray(W1, dtype=np.float32))
    W2 = np.ascontiguousarray(np.asarray(W2, dtype=np.float32))
    B, S, _ = x.shape
    xf = x.reshape(B * S, D)
    assert B * S == TOK_TOTAL

    if "nc" not in _CACHED:
        _CACHED["nc"] = build_program(TOK_PER_CORE)
    nc = _CACHED["nc"]

    consts = make_consts()
    in_maps = []
    for c in range(N_CORES):
        shard = np.ascontiguousarray(
            xf[c * TOK_PER_CORE:(c + 1) * TOK_PER_CORE])
        in_maps.append({"x": shard, "w1": W1, "w2": W2, **consts})

    trace = bool(int(os.environ.get("ROUTER_TRACE", "0")))
    res = bass_utils.run_bass_kernel_spmd(
        nc, in_maps, core_ids=list(range(N_CORES)), trace=trace)
    LAST_RESULTS = res

    scores = np.concatenate([r["scores"] for r in res.results], axis=0)
    idx = np.concatenate([r["idx"] for r in res.results], axis=0)
    gates = np.concatenate([r["gates"] for r in res.results], axis=0)
    return (idx.reshape(B, S, TOPK).astype(np.int32),
            gates.reshape(B, S, TOPK),
            scores.reshape(B, S, KOUT))


# revision 28
# speedup vs baseline: 1.4342x; 1.1251x over previous
"""MultiHeadProductKeyRouter Trainium2 Bass kernel.

Math: reference computes
    s1 = x @ W1.T  -> (T, H*SK) -> (T, H, SK)
    s2 = x @ W2.T
    head_scores[t,h,i*SK+j] = s1[t,h,i] + s2[t,h,j]
    scores = head_scores.mean(axis=-2)          # (T, SK*SK)
    topk_vals, topk_idx = top_k(scores, 8); gates = softmax(topk_vals)

Since the mean over heads commutes with the outer-sum:
    scores[t, i*SK+j] = a1[t,i] + a2[t,j]
with a1 = mean_h s1, a2 = mean_h s2, i.e. a = x @ Vall.T where
Vall = [mean_h W1_heads ; mean_h W2_heads]  (128 x 2048).

scores is produced on the TensorEngine as A @ M where A = [a1|a2] (128
features per token) and M is a constant 128x4096 0/1 matrix with
M[i, i*64+j] = 1 and M[64+j, i*64+j] = 1.  Each PSUM element is then a
single fp32 add a1[i]+a2[j] (zeros accumulate exactly), bit-identical to
the candidate sums used for top-k.

Top-8 of the outer sum: any top-8 element must have i in top8(a1) and
j in top8(a2), so top-8 over the 64 candidate sums of top8(a1) x top8(a2)
equals top-8 of the full 4096 row.  DVE max/max_index provide top-8
values+indices directly; candidate position is decoded as (p,q) = (c//8,
c%8) and mapped through the top-8 index lists.

Sharding: data parallel over tokens, 16384 tokens -> 8 cores x 2048.
"""

import os
import numpy as np

import concourse.bass as bass
import concourse.mybir as mybir
import concourse.tile as tile
from concourse import bass_utils

F32 = mybir.dt.float32
U32 = mybir.dt.uint32
I32 = mybir.dt.int32
Alu = mybir.AluOpType

D = 2048          # model dim
SK = 64           # sqrt(num experts)
KOUT = SK * SK    # 4096
NF = 128          # features after head-mean: 64 (a1) + 64 (a2)
TOPK = 8
DC = D // 128     # 16 contraction chunks
N_CORES = 8
TOK_TOTAL = 4 * 4096
TOK_PER_CORE = TOK_TOTAL // N_CORES   # 2048
GROUP = 512       # tokens per matmul1 group
SUB = GROUP // 128


# ---------------------------------------------------------------- host consts
def make_consts():
    # Head-sum selectors: W tiles are [128 rows = 2 heads x 64] ; out column
    # (i or 64+j) accumulates 0.25 * row (p % 64).
    hsel1 = np.zeros((128, 128), np.float32)
    hsel2 = np.zeros((128, 128), np.float32)
    for p in range(128):
        hsel1[p, p % SK] = 0.25
        hsel2[p, SK + (p % SK)] = 0.25
    ident = np.eye(128, dtype=np.float32)
    return {"hsel1": hsel1, "hsel2": hsel2, "ident": ident}


# Number of trailing i-blocks (of 8 rows of 64 scores each) of the outer-sum
# handed to GpSimd per subtile; rest goes to DVE.
GPSIMD_IBLOCKS = 0


def _bc_outer(a, m):
    """[128, n] -> [128, n, m] view repeating each element (stride-0 inner)."""
    return bass.AP(tensor=a.tensor, offset=a.offset,
                   ap=[a.ap[0], a.ap[1], [0, m]])


def _bc_inner(a, n):
    """[128, m] -> [128, n, m] view repeating the row n times (stride-0 outer)."""
    return bass.AP(tensor=a.tensor, offset=a.offset,
                   ap=[a.ap[0], [0, n], a.ap[1]])


def _r3(sc, m):
    return sc.rearrange("p (i j) -> p i j", j=m)


# ---------------------------------------------------------------- tile kernel
def router_tile_kernel(tc, outs, ins, n_tokens, ctx):
    nc = tc.nc
    n_groups = n_tokens // GROUP

    x = ins["x"]            # [n_tokens, D]
    w1 = ins["w1"]          # [256, D]
    w2 = ins["w2"]          # [256, D]
    hsel1 = ins["hsel1"]    # [128, 128]
    hsel2 = ins["hsel2"]    # [128, 128]
    ident = ins["ident"]    # [128, 128]
    scores_d = outs["scores"]   # [n_tokens, KOUT] f32
    idx_d = outs["idx"]         # [n_tokens, TOPK] i32
    gates_d = outs["gates"]     # [n_tokens, TOPK] f32

    consts = ctx.enter_context(tc.tile_pool(name="consts", bufs=1))
    xpool = ctx.enter_context(tc.tile_pool(name="xpool", bufs=5))
    wpool = xpool
    xtpool = ctx.enter_context(tc.tile_pool(name="xtpool", bufs=2))
    apool = ctx.enter_context(tc.tile_pool(name="apool", bufs=2))
    atpool = ctx.enter_context(tc.tile_pool(name="atpool", bufs=2))
    spool = ctx.enter_context(tc.tile_pool(name="spool", bufs=2))
    small = ctx.enter_context(tc.tile_pool(name="small", bufs=3))
    ps_tr = ctx.enter_context(tc.tile_pool(name="ps_tr", bufs=4, space="PSUM"))
    ps_mm1 = ctx.enter_context(tc.tile_pool(name="ps_mm1", bufs=2, space="PSUM"))
    ps_at = ctx.enter_context(tc.tile_pool(name="ps_at", bufs=2, space="PSUM"))

    # ---- constants into SBUF
    hs1_sb = consts.tile([128, 128], F32)
    nc.sync.dma_start(hs1_sb, hsel1)
    hs2_sb = consts.tile([128, 128], F32)
    nc.sync.dma_start(hs2_sb, hsel2)
    id_sb = consts.tile([128, 128], F32)
    nc.sync.dma_start(id_sb, ident)

    # ---- Vall = 0.25 * head-sum of [W1;W2]  -> [128 f, D]
    vall_sb = consts.tile([128, D], F32)
    pv = [ps_tr.tile([128, 512], F32, tag="tr", name=f"pv{c4}")
          for c4 in range(D // 512)]
    for i, (w, hs) in enumerate(
            ((w1, hs1_sb), (w1, hs1_sb), (w2, hs2_sb), (w2, hs2_sb))):
        r = i % 2
        wt = wpool.tile([128, D], F32, tag="x")
        nc.sync.dma_start(wt, w[r * 128:(r + 1) * 128, :])
        for c4 in range(D // 512):
            nc.tensor.matmul(pv[c4], hs, wt[:, c4 * 512:(c4 + 1) * 512],
                             start=(i == 0), stop=(i == 3))
    for c4 in range(D // 512):
        nc.vector.tensor_copy(vall_sb[:, c4 * 512:(c4 + 1) * 512], pv[c4])

    # ---- VallT chunks [128 d, 128 f] for matmul1 stationary
    vallT_sb = consts.tile([128, DC, 128], F32)
    for c4 in range(DC // 4):
        pt = ps_tr.tile([128, 512], F32, tag="tr")
        for cc in range(4):
            c = c4 * 4 + cc
            nc.tensor.transpose(pt[:, cc * 128:(cc + 1) * 128],
                                vall_sb[:, c * 128:(c + 1) * 128], id_sb)
        nc.vector.tensor_copy(vallT_sb[:, c4 * 4:(c4 + 1) * 4, :], pt)

    # ---- main loop over token groups
    for g in range(n_groups):
        xt = xtpool.tile([128, DC, GROUP], F32, tag="xt")  # [d-chunk part, c, t]
        for s in range(SUB):
            xs = xpool.tile([128, D], F32, tag="x")
            # x loads ride the ACT HWDGE ring so they are not FIFO-ordered
            # behind the large score stores on the SP ring
            nc.scalar.dma_start(xs, x[(g * SUB + s) * 128:(g * SUB + s + 1) * 128, :])
            for c4 in range(DC // 4):
                pt = ps_tr.tile([128, 512], F32, tag="tr")
                for cc in range(4):
                    c = c4 * 4 + cc
                    nc.tensor.transpose(pt[:, cc * 128:(cc + 1) * 128],
                                        xs[:, c * 128:(c + 1) * 128], id_sb)
                # one strided copy: psum [128,4,128] -> xt[:, 4c4:4c4+4, s*128:+128]
                dst = xt[:, c4 * 4:(c4 + 1) * 4, s * 128:(s + 1) * 128]
                nc.scalar.copy(dst, pt.rearrange("p (c t) -> p c t", c=4))

        # matmul1: A[f, t] = sum_c VallT[c].T @ xt[c]
        a_ps = ps_mm1.tile([128, GROUP], F32, tag="mm1")
        for c in range(DC):
            nc.tensor.matmul(a_ps, vallT_sb[:, c, :], xt[:, c, :],
                             start=(c == 0), stop=(c == DC - 1))
        a_sb = apool.tile([128, GROUP], F32, tag="a")
        nc.scalar.copy(a_sb, a_ps)

        # At[t, f] per subtile (for top-k)
        at_ps = ps_at.tile([128, SUB, 128], F32, tag="at")
        for s in range(SUB):
            nc.tensor.transpose(at_ps[:, s, :], a_sb[:, s * 128:(s + 1) * 128], id_sb)
        at_sb = atpool.tile([128, SUB, 128], F32, tag="at")
        nc.scalar.copy(at_sb, at_ps)

        # scores outer-sum + topk per subtile
        for s in range(SUB):
            tok0 = (g * SUB + s) * 128
            at = at_sb[:, s, :]
            a1 = at[:, 0:SK]
            a2 = at[:, SK:2 * SK]
            sc_sb = spool.tile([128, KOUT], F32, tag="sc")
            # scores[t, i*64+j] = a1[t,i] + a2[t,j] via stride-0 broadcast APs
            if GPSIMD_IBLOCKS:
                split = SK - GPSIMD_IBLOCKS * 8
                nc.vector.tensor_add(
                    _r3(sc_sb[:, :split * SK], SK),
                    _bc_outer(a1[:, :split], SK), _bc_inner(a2, split))
                nc.gpsimd.tensor_add(
                    _r3(sc_sb[:, split * SK:], SK),
                    _bc_outer(a1[:, split:], SK), _bc_inner(a2, SK - split))
            else:
                nc.vector.tensor_add(
                    _r3(sc_sb, SK), _bc_outer(a1, SK), _bc_inner(a2, SK))
            nc.sync.dma_start(scores_d[tok0:tok0 + 128, :], sc_sb)

            _topk_block(nc, small, at, idx_d, gates_d, tok0)


def _topk_block(nc, small, at, idx_d, gates_d, tok0):
    """Top-8 + gates for one 128-token subtile. at: [128 t, 128 f] SBUF."""
    a1 = at[:, 0:SK]
    a2 = at[:, SK:2 * SK]
    t8 = [128, TOPK]

    v1 = small.tile(t8, F32, tag="v1")
    v2 = small.tile(t8, F32, tag="v2")
    i1u = small.tile(t8, U32, tag="i1u")
    i2u = small.tile(t8, U32, tag="i2u")
    i1f = small.tile(t8, F32, tag="i1f")
    i2f = small.tile(t8, F32, tag="i2f")
    i1n = small.tile(t8, F32, tag="i1n")
    cv = small.tile([128, 64], F32, tag="cv")
    cin = small.tile([128, 64], F32, tag="cin")
    tv = small.tile(t8, F32, tag="tv")
    eqm = small.tile([128, TOPK, 64], F32, tag="eqm")
    red = small.tile(t8, F32, tag="red")
    flatf = small.tile(t8, F32, tag="flatf")
    idx_sb = small.tile(t8, I32, tag="idx_sb")
    negm = small.tile([128, 1], F32, tag="negm")
    e8 = small.tile(t8, F32, tag="e8")
    ssum = small.tile([128, 1], F32, tag="ssum")
    rinv = small.tile([128, 1], F32, tag="rinv")
    gates_sb = small.tile(t8, F32, tag="gates_sb")

    # top-8 of each half (values sorted desc; indices = first occurrence)
    nc.vector.max(v1, a1)
    nc.vector.max_index(i1u, v1, a1)
    nc.vector.max(v2, a2)
    nc.vector.max_index(i2u, v2, a2)
    nc.vector.tensor_copy(i1f, i1u)
    nc.vector.tensor_copy(i2f, i2u)

    # 64 candidates cv[p*8+q] = v1[p] + v2[q]; the exact same fp32 adds as
    # the scores outer-sum, so values are bit-identical to scores entries.
    nc.vector.tensor_add(_r3(cv, 8), _bc_outer(v1, 8), _bc_inner(v2, 8))
    # negated flat index per candidate: cin[p*8+q] = 4096 - (64*i1[p] + i2[q])
    nc.vector.tensor_scalar(i1n, i1f, -64.0, 4096.0, op0=Alu.mult, op1=Alu.add)
    nc.vector.tensor_tensor(_r3(cin, 8), _bc_outer(i1n, 8), _bc_inner(i2f, 8),
                            op=Alu.subtract)

    nc.vector.max(tv, cv)

    # flat idx of the k-th winner: match tv[k] against cv, pick the matching
    # candidate's min flat idx (max of cin), all in one masked reduce.
    nc.vector.tensor_tensor(eqm, _bc_inner(cv, 8),
                            _bc_outer(tv, 64), op=Alu.is_equal)
    nc.vector.tensor_mul(eqm, eqm, _bc_inner(cin, 8))
    nc.vector.reduce_max(red, eqm, axis=mybir.AxisListType.X)
    nc.vector.tensor_scalar(flatf, red, -1.0, 4096.0, op0=Alu.mult, op1=Alu.add)
    nc.vector.tensor_copy(idx_sb, flatf)

    # gates = softmax(tv) (tv[:,0] is the max)
    nc.vector.tensor_scalar_mul(negm, tv[:, 0:1], -1.0)
    nc.scalar.activation(e8, tv, mybir.ActivationFunctionType.Exp,
                         bias=negm, scale=1.0, accum_out=ssum)
    nc.vector.reciprocal(rinv, ssum)
    nc.vector.tensor_scalar_mul(gates_sb, e8, rinv)

    nc.sync.dma_start(idx_d[tok0:tok0 + 128, :], idx_sb)
    nc.sync.dma_start(gates_d[tok0:tok0 + 128, :], gates_sb)


def _split_multi_waits(nc, limit=1):
    """Walrus in this toolchain rejects instructions carrying more than one
    semaphore wait (fp32 Matmult LDW path asserts at even 2, end-of-kernel
    drains at 5).  Post-process the scheduled BIR: move all but `limit`
    waits of each instruction onto same-engine no-ops inserted right before
    it.  Engine-level serialization keeps the semantics identical."""
    k = 0
    for f in nc.m.functions:
        for b in f.blocks:
            out = []
            changed = False
            for inst in b.instructions:
                si = inst.sync_info
                if si is not None and si.on_wait and len(si.on_wait) > limit:
                    waits = list(si.on_wait)
                    for w in waits[:-limit]:
                        nop = mybir.InstNoOp(name=f"I-nw{k}", ins=[], outs=[])
                        k += 1
                        nop.engine = inst.engine
                        nop.sync_info = mybir.SyncInfo(on_wait=[w], on_update=[])
                        out.append(nop)
                    inst.sync_info = mybir.SyncInfo(
                        on_wait=waits[-limit:], on_update=list(si.on_update))
                    changed = True
                out.append(inst)
            if changed:
                b.instructions = out


# ---------------------------------------------------------------- program
def build_program(n_tokens=TOK_PER_CORE, split_waits=True):
    from contextlib import ExitStack
    nc = bass.Bass("TRN2", target_bir_lowering=False, debug=False,
                   num_devices=N_CORES)
    ins = {
        "x": nc.dram_tensor("x", [n_tokens, D], F32, kind="ExternalInput").ap(),
        "w1": nc.dram_tensor("w1", [4 * SK, D], F32, kind="ExternalInput").ap(),
        "w2": nc.dram_tensor("w2", [4 * SK, D], F32, kind="ExternalInput").ap(),
        "hsel1": nc.dram_tensor("hsel1", [128, 128], F32, kind="ExternalInput").ap(),
        "hsel2": nc.dram_tensor("hsel2", [128, 128], F32, kind="ExternalInput").ap(),
        "ident": nc.dram_tensor("ident", [128, 128], F32, kind="ExternalInput").ap(),
    }
    outs = {
        "scores": nc.dram_tensor("scores", [n_tokens, KOUT], F32,
                                 kind="ExternalOutput").ap(),
        "idx": nc.dram_tensor("idx", [n_tokens, TOPK], I32,
                              kind="ExternalOutput").ap(),
        "gates": nc.dram_tensor("gates", [n_tokens, TOPK], F32,
                                kind="ExternalOutput").ap(),
    }
    with tile.TileContext(nc) as tc:
        with ExitStack() as ctx:
            router_tile_kernel(tc, outs, ins, n_tokens, ctx)
    if split_waits:
        _split_multi_waits(nc)
    return nc


_CACHED = {}
LAST_RESULTS = None


def kernel(x, W1, W2):
    """Full-input entry point: shards tokens over 8 cores, returns full outputs."""
    global LAST_RESULTS
    x = np.ascontiguousarray(np.asarray(x, dtype=np.float32))
    W1 = np.ascontiguousarray(np.asarray(W1, dtype=np.float32))
    W2 = np.ascontiguousarray(np.asarray(W2, dtype=np.float32))
    B, S, _ = x.shape
    xf = x.reshape(B * S, D)
    assert B * S == TOK_TOTAL

    if "nc" not in _CACHED:
        _CACHED["nc"] = build_program(TOK_PER_CORE)
    nc = _CACHED["nc"]

    consts = make_consts()
    in_maps = []
    for c in range(N_CORES):
        shard = np.ascontiguousarray(
            xf[c * TOK_PER_CORE:(c + 1) * TOK_PER_CORE])
        in_maps.append({"x": shard, "w1": W1, "w2": W2, **consts})

    trace = bool(int(os.environ.get("ROUTER_TRACE", "0")))
    res = bass_utils.run_bass_kernel_spmd(
        nc, in_maps, core_ids=list(range(N_CORES)), trace=trace)
    LAST_RESULTS = res

    scores = np.concatenate([r["scores"] for r in res.results], axis=0)
    idx = np.concatenate([r["idx"] for r in res.results], axis=0)
    gates = np.concatenate([r["gates"] for r in res.results], axis=0)
    return (idx.reshape(B, S, TOPK).astype(np.int32),
            gates.reshape(B, S, TOPK),
            scores.reshape(B, S, KOUT))


# revision 36
# speedup vs baseline: 1.5263x; 1.0642x over previous
"""MultiHeadProductKeyRouter Trainium2 Bass kernel.

Math: reference computes
    s1 = x @ W1.T  -> (T, H*SK) -> (T, H, SK)
    s2 = x @ W2.T
    head_scores[t,h,i*SK+j] = s1[t,h,i] + s2[t,h,j]
    scores = head_scores.mean(axis=-2)          # (T, SK*SK)
    topk_vals, topk_idx = top_k(scores, 8); gates = softmax(topk_vals)

Since the mean over heads commutes with the outer-sum:
    scores[t, i*SK+j] = a1[t,i] + a2[t,j]
with a1 = mean_h s1, a2 = mean_h s2, i.e. a = x @ Vall.T where
Vall = [mean_h W1_heads ; mean_h W2_heads]  (128 x 2048).

scores is produced on the TensorEngine as A @ M where A = [a1|a2] (128
features per token) and M is a constant 128x4096 0/1 matrix with
M[i, i*64+j] = 1 and M[64+j, i*64+j] = 1.  Each PSUM element is then a
single fp32 add a1[i]+a2[j] (zeros accumulate exactly), bit-identical to
the candidate sums used for top-k.

Top-8 of the outer sum: any top-8 element must have i in top8(a1) and
j in top8(a2), so top-8 over the 64 candidate sums of top8(a1) x top8(a2)
equals top-8 of the full 4096 row.  DVE max/max_index provide top-8
values+indices directly; candidate position is decoded as (p,q) = (c//8,
c%8) and mapped through the top-8 index lists.

Sharding: data parallel over tokens, 16384 tokens -> 8 cores x 2048.
"""

import os
import numpy as np

import concourse.bass as bass
import concourse.mybir as mybir
import concourse.tile as tile
from concourse import bass_utils

F32 = mybir.dt.float32
U32 = mybir.dt.uint32
I32 = mybir.dt.int32
Alu = mybir.AluOpType

D = 2048          # model dim
SK = 64           # sqrt(num experts)
KOUT = SK * SK    # 4096
NF = 128          # features after head-mean: 64 (a1) + 64 (a2)
TOPK = 8
DC = D // 128     # 16 contraction chunks
N_CORES = 8
TOK_TOTAL = 4 * 4096
TOK_PER_CORE = TOK_TOTAL // N_CORES   # 2048
GROUP = 512       # tokens per matmul1 group
SUB = GROUP // 128


# ---------------------------------------------------------------- host consts
def make_consts():
    ident = np.eye(128, dtype=np.float32)
    return {"ident": ident}


# Number of trailing i-blocks (of 8 rows of 64 scores each) of the outer-sum
# handed to GpSimd per subtile; rest goes to DVE.
GPSIMD_IBLOCKS = 0


def _bc_outer(a, m):
    """[128, n] -> [128, n, m] view repeating each element (stride-0 inner)."""
    return bass.AP(tensor=a.tensor, offset=a.offset,
                   ap=[a.ap[0], a.ap[1], [0, m]])


def _bc_inner(a, n):
    """[128, m] -> [128, n, m] view repeating the row n times (stride-0 outer)."""
    return bass.AP(tensor=a.tensor, offset=a.offset,
                   ap=[a.ap[0], [0, n], a.ap[1]])


def _r3(sc, m):
    return sc.rearrange("p (i j) -> p i j", j=m)


# ---------------------------------------------------------------- tile kernel
def router_tile_kernel(tc, outs, ins, n_tokens, ctx):
    nc = tc.nc
    n_groups = n_tokens // GROUP

    x = ins["x"]            # [n_tokens, D]
    w1t = ins["w1t"]        # [D, 256]  (W1 transposed on host; layout only)
    w2t = ins["w2t"]        # [D, 256]
    ident = ins["ident"]    # [128, 128]
    scores_d = outs["scores"]   # [n_tokens, KOUT] f32
    idx_d = outs["idx"]         # [n_tokens, TOPK] i32
    gates_d = outs["gates"]     # [n_tokens, TOPK] f32

    consts = ctx.enter_context(tc.tile_pool(name="consts", bufs=1))
    xpool = ctx.enter_context(tc.tile_pool(name="xpool", bufs=4))
    wpool = ctx.enter_context(tc.tile_pool(name="wpool", bufs=2))
    xtpool = ctx.enter_context(tc.tile_pool(name="xtpool", bufs=2))
    apool = ctx.enter_context(tc.tile_pool(name="apool", bufs=2))
    atpool = ctx.enter_context(tc.tile_pool(name="atpool", bufs=2))
    spool = ctx.enter_context(tc.tile_pool(name="spool", bufs=2))
    small = ctx.enter_context(tc.tile_pool(name="small", bufs=3))
    ps_tr = ctx.enter_context(tc.tile_pool(name="ps_tr", bufs=4, space="PSUM"))
    ps_mm1 = ctx.enter_context(tc.tile_pool(name="ps_mm1", bufs=2, space="PSUM"))
    ps_at = ctx.enter_context(tc.tile_pool(name="ps_at", bufs=2, space="PSUM"))

    # ---- constants into SBUF
    id_sb = consts.tile([128, 128], F32)
    nc.sync.dma_start(id_sb, ident)

    # ---- VallT[d, 0:64] = mean_h W1T heads, VallT[d, 64:128] = mean_h W2T.
    # W arrives host-transposed [D, 256] so the head-mean is a free-dim add.
    vallT_sb = consts.tile([128, DC, 128], F32)
    for wi, (wt_d, base) in enumerate(((w1t, 0), (w2t, SK))):
        wt_sb = wpool.tile([128, DC, 4 * SK], F32, tag="wt")
        nc.sync.dma_start(wt_sb, wt_d.rearrange("(c p) e -> p c e", p=128))
        t1 = wpool.tile([128, DC, SK], F32, tag="t1", bufs=1)
        t2 = wpool.tile([128, DC, SK], F32, tag="t2", bufs=1)
        nc.vector.tensor_add(t1, wt_sb[:, :, 0:SK], wt_sb[:, :, SK:2 * SK])
        nc.vector.tensor_add(t2, wt_sb[:, :, 2 * SK:3 * SK], wt_sb[:, :, 3 * SK:])
        nc.vector.tensor_add(t1, t1, t2)
        nc.vector.tensor_scalar_mul(vallT_sb[:, :, base:base + SK], t1, 0.25)

    # ---- main loop over token groups
    for g in range(n_groups):
        xt = xtpool.tile([128, DC, GROUP], F32, tag="xt")  # [d-chunk part, c, t]
        for s in range(SUB):
            xs = xpool.tile([128, D], F32, tag="x")
            # x loads ride the ACT HWDGE ring so they are not FIFO-ordered
            # behind the large score stores on the SP ring
            nc.scalar.dma_start(xs, x[(g * SUB + s) * 128:(g * SUB + s + 1) * 128, :])
            for c4 in range(DC // 4):
                pt = ps_tr.tile([128, 512], F32, tag="tr")
                for cc in range(4):
                    c = c4 * 4 + cc
                    nc.tensor.transpose(pt[:, cc * 128:(cc + 1) * 128],
                                        xs[:, c * 128:(c + 1) * 128], id_sb)
                # one strided copy: psum [128,4,128] -> xt[:, 4c4:4c4+4, s*128:+128]
                dst = xt[:, c4 * 4:(c4 + 1) * 4, s * 128:(s + 1) * 128]
                nc.scalar.copy(dst, pt.rearrange("p (c t) -> p c t", c=4))

        # matmul1: A[f, t] = sum_c VallT[c].T @ xt[c]
        a_ps = ps_mm1.tile([128, GROUP], F32, tag="mm1")
        for c in range(DC):
            nc.tensor.matmul(a_ps, vallT_sb[:, c, :], xt[:, c, :],
                             start=(c == 0), stop=(c == DC - 1))
        a_sb = apool.tile([128, GROUP], F32, tag="a")
        nc.scalar.copy(a_sb, a_ps)

        # At[t, f] per subtile (for top-k)
        at_ps = ps_at.tile([128, SUB, 128], F32, tag="at")
        for s in range(SUB):
            nc.tensor.transpose(at_ps[:, s, :], a_sb[:, s * 128:(s + 1) * 128], id_sb)
        at_sb = atpool.tile([128, SUB, 128], F32, tag="at")
        nc.scalar.copy(at_sb, at_ps)

        # scores outer-sum + topk per subtile
        for s in range(SUB):
            tok0 = (g * SUB + s) * 128
            at = at_sb[:, s, :]
            a1 = at[:, 0:SK]
            a2 = at[:, SK:2 * SK]
            sc_sb = spool.tile([128, KOUT], F32, tag="sc")
            # scores[t, i*64+j] = a1[t,i] + a2[t,j] via stride-0 broadcast APs
            if GPSIMD_IBLOCKS:
                split = SK - GPSIMD_IBLOCKS * 8
                nc.vector.tensor_add(
                    _r3(sc_sb[:, :split * SK], SK),
                    _bc_outer(a1[:, :split], SK), _bc_inner(a2, split))
                nc.gpsimd.tensor_add(
                    _r3(sc_sb[:, split * SK:], SK),
                    _bc_outer(a1[:, split:], SK), _bc_inner(a2, SK - split))
            else:
                nc.vector.tensor_add(
                    _r3(sc_sb, SK), _bc_outer(a1, SK), _bc_inner(a2, SK))
            nc.sync.dma_start(scores_d[tok0:tok0 + 128, :], sc_sb)

            _topk_block(nc, small, at, idx_d, gates_d, tok0)


def _topk_block(nc, small, at, idx_d, gates_d, tok0):
    """Top-8 + gates for one 128-token subtile. at: [128 t, 128 f] SBUF."""
    a1 = at[:, 0:SK]
    a2 = at[:, SK:2 * SK]
    t8 = [128, TOPK]

    v1 = small.tile(t8, F32, tag="v1")
    v2 = small.tile(t8, F32, tag="v2")
    i1u = small.tile(t8, U32, tag="i1u")
    i2u = small.tile(t8, U32, tag="i2u")
    i1f = small.tile(t8, F32, tag="i1f")
    i2f = small.tile(t8, F32, tag="i2f")
    i1n = small.tile(t8, F32, tag="i1n")
    cv = small.tile([128, 64], F32, tag="cv")
    cin = small.tile([128, 64], F32, tag="cin")
    tv = small.tile(t8, F32, tag="tv")
    eqm = small.tile([128, TOPK, 64], F32, tag="eqm")
    red = small.tile(t8, F32, tag="red")
    flatf = small.tile(t8, F32, tag="flatf")
    idx_sb = small.tile(t8, I32, tag="idx_sb")
    negm = small.tile([128, 1], F32, tag="negm")
    e8 = small.tile(t8, F32, tag="e8")
    ssum = small.tile([128, 1], F32, tag="ssum")
    rinv = small.tile([128, 1], F32, tag="rinv")
    gates_sb = small.tile(t8, F32, tag="gates_sb")

    # top-8 of each half (values sorted desc; indices = first occurrence)
    nc.vector.max(v1, a1)
    nc.vector.max_index(i1u, v1, a1)
    nc.vector.max(v2, a2)
    nc.vector.max_index(i2u, v2, a2)
    nc.vector.tensor_copy(i1f, i1u)
    nc.vector.tensor_copy(i2f, i2u)

    # 64 candidates cv[p*8+q] = v1[p] + v2[q]; the exact same fp32 adds as
    # the scores outer-sum, so values are bit-identical to scores entries.
    nc.vector.tensor_add(_r3(cv, 8), _bc_outer(v1, 8), _bc_inner(v2, 8))
    # negated flat index per candidate: cin[p*8+q] = 4096 - (64*i1[p] + i2[q])
    nc.vector.tensor_scalar(i1n, i1f, -64.0, 4096.0, op0=Alu.mult, op1=Alu.add)
    nc.vector.tensor_tensor(_r3(cin, 8), _bc_outer(i1n, 8), _bc_inner(i2f, 8),
                            op=Alu.subtract)

    nc.vector.max(tv, cv)

    # flat idx of the k-th winner: match tv[k] against cv, pick the matching
    # candidate's min flat idx (max of cin), all in one masked reduce.
    nc.vector.tensor_tensor(eqm, _bc_inner(cv, 8),
                            _bc_outer(tv, 64), op=Alu.is_equal)
    nc.vector.tensor_mul(eqm, eqm, _bc_inner(cin, 8))
    nc.vector.reduce_max(red, eqm, axis=mybir.AxisListType.X)
    nc.vector.tensor_scalar(flatf, red, -1.0, 4096.0, op0=Alu.mult, op1=Alu.add)
    nc.vector.tensor_copy(idx_sb, flatf)

    # gates = softmax(tv) (tv[:,0] is the max)
    nc.vector.tensor_scalar_mul(negm, tv[:, 0:1], -1.0)
    nc.scalar.activation(e8, tv, mybir.ActivationFunctionType.Exp,
                         bias=negm, scale=1.0, accum_out=ssum)
    nc.vector.reciprocal(rinv, ssum)
    nc.vector.tensor_scalar_mul(gates_sb, e8, rinv)

    nc.sync.dma_start(idx_d[tok0:tok0 + 128, :], idx_sb)
    nc.sync.dma_start(gates_d[tok0:tok0 + 128, :], gates_sb)


def _split_multi_waits(nc, limit=1):
    """Walrus in this toolchain rejects instructions carrying more than one
    semaphore wait (fp32 Matmult LDW path asserts at even 2, end-of-kernel
    drains at 5).  Post-process the scheduled BIR: move all but `limit`
    waits of each instruction onto same-engine no-ops inserted right before
    it.  Engine-level serialization keeps the semantics identical."""
    k = 0
    for f in nc.m.functions:
        for b in f.blocks:
            out = []
            changed = False
            for inst in b.instructions:
                si = inst.sync_info
                if si is not None and si.on_wait and len(si.on_wait) > limit:
                    waits = list(si.on_wait)
                    for w in waits[:-limit]:
                        nop = mybir.InstNoOp(name=f"I-nw{k}", ins=[], outs=[])
                        k += 1
                        nop.engine = inst.engine
                        nop.sync_info = mybir.SyncInfo(on_wait=[w], on_update=[])
                        out.append(nop)
                    inst.sync_info = mybir.SyncInfo(
                        on_wait=waits[-limit:], on_update=list(si.on_update))
                    changed = True
                out.append(inst)
            if changed:
                b.instructions = out


# ---------------------------------------------------------------- program
def build_program(n_tokens=TOK_PER_CORE, split_waits=True):
    from contextlib import ExitStack
    nc = bass.Bass("TRN2", target_bir_lowering=False, debug=False,
                   num_devices=N_CORES)
    ins = {
        "x": nc.dram_tensor("x", [n_tokens, D], F32, kind="ExternalInput").ap(),
        "w1t": nc.dram_tensor("w1t", [D, 4 * SK], F32, kind="ExternalInput").ap(),
        "w2t": nc.dram_tensor("w2t", [D, 4 * SK], F32, kind="ExternalInput").ap(),
        "ident": nc.dram_tensor("ident", [128, 128], F32, kind="ExternalInput").ap(),
    }
    outs = {
        "scores": nc.dram_tensor("scores", [n_tokens, KOUT], F32,
                                 kind="ExternalOutput").ap(),
        "idx": nc.dram_tensor("idx", [n_tokens, TOPK], I32,
                              kind="ExternalOutput").ap(),
        "gates": nc.dram_tensor("gates", [n_tokens, TOPK], F32,
                                kind="ExternalOutput").ap(),
    }
    with tile.TileContext(nc) as tc:
        with ExitStack() as ctx:
            router_tile_kernel(tc, outs, ins, n_tokens, ctx)
    if split_waits:
        _split_multi_waits(nc)
    return nc


_CACHED = {}
LAST_RESULTS = None


def kernel(x, W1, W2):
    """Full-input entry point: shards tokens over 8 cores, returns full outputs."""
    global LAST_RESULTS
    x = np.ascontiguousarray(np.asarray(x, dtype=np.float32))
    W1 = np.ascontiguousarray(np.asarray(W1, dtype=np.float32))
    W2 = np.ascontiguousarray(np.asarray(W2, dtype=np.float32))
    B, S, _ = x.shape
    xf = x.reshape(B * S, D)
    assert B * S == TOK_TOTAL

    if "nc" not in _CACHED:
        _CACHED["nc"] = build_program(TOK_PER_CORE)
    nc = _CACHED["nc"]

    consts = make_consts()
    w1t = np.ascontiguousarray(W1.T)
    w2t = np.ascontiguousarray(W2.T)
    in_maps = []
    for c in range(N_CORES):
        shard = np.ascontiguousarray(
            xf[c * TOK_PER_CORE:(c + 1) * TOK_PER_CORE])
        in_maps.append({"x": shard, "w1t": w1t, "w2t": w2t, **consts})

    trace = bool(int(os.environ.get("ROUTER_TRACE", "0")))
    res = bass_utils.run_bass_kernel_spmd(
        nc, in_maps, core_ids=list(range(N_CORES)), trace=trace)
    LAST_RESULTS = res

    scores = np.concatenate([r["scores"] for r in res.results], axis=0)
    idx = np.concatenate([r["idx"] for r in res.results], axis=0)
    gates = np.concatenate([r["gates"] for r in res.results], axis=0)
    return (idx.reshape(B, S, TOPK).astype(np.int32),
            gates.reshape(B, S, TOPK),
            scores.reshape(B, S, KOUT))
